# revision 2
# baseline (speedup 1.0000x reference)
"""MoE grouped linear (DMoELinear) on 8 Trainium2 NeuronCores.

Expert-parallel sharding: tokens are sorted by expert id, so expert e's
tokens form one contiguous slice. Core e receives expert e's tokens
(padded to a uniform capacity C = max group size, so all cores run one
SPMD NEFF), expert e's weight and bias, and computes
    yT_e = (x_e @ W_e.T).T.bf16 + b_e.bf16
with the weight block as the stationary matmul operand and tokens as
the moving free dim. Structure (per core):

  main phase: token columns [0, 1024) as two 512-col PSUM banks per
    output block. The first wave interleaves output blocks 0-2 per
    k-step (3 blocks x 2 banks = 6 PSUM banks) so the PE consumes each
    freshly-landed x k-tile three times (~1.3us) while the next tile
    streams in (~0.9us) - the PE never starves during the x trickle.
    Blocks 3..15 then run one at a time.
  leftover phase: columns [1024, C) for all 16 output blocks at the
    end, in two 8-bank sub-waves, each evacuated into one contiguous
    SBUF strip and written with a single small DMA - keeps the final
    compute->evac->DMA->drain chain short.

Input DMAs are emitted in PE-consumption ("need time") order via a
greedy two-ring assignment: tiny per-k-slice pieces of the first three
weight blocks and the x tiles lead, bulk weight blocks follow. The
bias add fuses into the PSUM-evacuation op as a per-partition scalar.
The host does all routing/gather in numpy.
"""

import numpy as np
import ml_dtypes

N_TOK, D_IN, D_OUT, N_EXP = 8192, 1024, 2048, 8
N_CORES = 8
P = 128
NFREE = 512  # max matmul moving free dim (one PSUM bank of f32)

BF16 = ml_dtypes.bfloat16

_nc_cache: dict[int, object] = {}

KT = D_IN // P   # 8 contraction tiles
DB = D_OUT // P  # 16 output-row blocks
WAVE0 = 3        # output blocks interleaved during the x stream-in


def _plan_dmas(C):
    """Greedy two-ring schedule of input pieces by PE need-time.

    Returns list of (ring, kind, args) in per-ring emission order merged
    (ring 0 = sync, ring 1 = scalar). Time unit: us. Model: 150 GB/s per
    ring + 0.15us per-submit overhead; PE consumes 1024 cycles (0.43us)
    per (db, ki) once data is there.
    """
    bw = 150e3  # bytes per us
    sub = 0.15
    kper = 0.43 * WAVE0  # us per ki step in wave0
    items = []  # (need_us, bytes, kind, args)
    # x0 split at the chunk boundary so the very first matmul only waits
    # for 512 columns; remaining x tiles whole.
    c0 = min(NFREE, C)
    items.append((0.0, P * c0 * 2, "x", (0, 0, c0)))
    if C > c0:
        items.append((0.1, P * (C - c0) * 2, "x", (0, c0, C)))
    for ki in range(1, KT):
        items.append((kper * ki, P * C * 2, "x", (ki, 0, C)))
    # first-wave weight blocks in per-ki pieces: [k0], [k1], [k2,k3], [k4:]
    for db in range(WAVE0):
        for (ka, kb) in ((0, 1), (1, 2), (2, 4), (4, KT)):
            items.append(
                (kper * ka + 0.43 * db, P * (kb - ka) * P * 2, "w", (db, ka, kb))
            )
    # remaining blocks whole, needed when their turn comes
    t_wave0 = kper * KT
    for db in range(WAVE0, DB):
        items.append(
            (t_wave0 + (db - WAVE0) * 0.43 * 2, P * D_IN * 2, "w", (db, 0, KT))
        )
    items.append((t_wave0, P * DB * 4, "bias", ()))

    items.sort(key=lambda it: it[0])
    ring_t = [0.0, 0.0]
    order = []
    worst = 0.0
    for need, nbytes, kind, args in items:
        r = 0 if ring_t[0] <= ring_t[1] else 1
        ring_t[r] += sub + nbytes / bw
        worst = max(worst, ring_t[r] - max(need, 0.5))
        order.append((r, kind, args))
    return order, worst


def _build_bass(C: int):
    """Emit the per-core Bass/Tile kernel for token capacity C."""
    import concourse.bass as bass  # noqa: F401  (registers engines)
    import concourse.mybir as mybir
    import concourse.tile as tile
    from concourse import bacc

    dt = mybir.dt
    C2 = min(C, 1024)          # main-phase columns (2 psum banks)
    CL = C - C2                # leftover columns
    chunks = [(0, min(NFREE, C2))]
    if C2 > NFREE:
        chunks.append((NFREE, C2 - NFREE))

    nc = bacc.Bacc("TRN2", target_bir_lowering=False)

    xT_d = nc.dram_tensor("xT", [D_IN, C], dt.bfloat16, kind="ExternalInput")
    # w: db-major, partition-contiguous: row (db*128+p) holds all KT
    # 128-wide k-slices for output block db, SBUF partition p.
    w_d = nc.dram_tensor("wdb", [DB * P, KT * P], dt.bfloat16, kind="ExternalInput")
    bias_d = nc.dram_tensor("biasp", [P, DB], dt.float32, kind="ExternalInput")
    # y layout [p, db, c] so one DMA can write a multi-db strip with
    # matching (partition, db, col) iteration order on both sides.
    y_d = nc.dram_tensor("yT", [P, DB, C], dt.bfloat16, kind="ExternalOutput")

    order, _ = _plan_dmas(C)

    with tile.TileContext(nc) as tc:
        with (
            tc.tile_pool(name="persist", bufs=1) as ppool,
            tc.tile_pool(name="yout", bufs=3) as ypool,
            tc.tile_pool(name="psum", bufs=8, space="PSUM") as pspool,
        ):
            x_tiles = [
                ppool.tile([P, C], dt.bfloat16, name=f"x{ki}", tag=f"x{ki}")
                for ki in range(KT)
            ]
            w_tiles = [
                ppool.tile([P, KT * P], dt.bfloat16, name=f"w{db}", tag=f"w{db}")
                for db in range(DB)
            ]
            bt = ppool.tile([P, DB], dt.float32, name="bias", tag="bias")

            for r, kind, args in order:
                eng = nc.sync if r == 0 else nc.scalar
                if kind == "x":
                    ki, ca, cb = args
                    eng.dma_start(
                        x_tiles[ki][:, ca:cb], xT_d[ki * P:(ki + 1) * P, ca:cb]
                    )
                elif kind == "w":
                    db, ka, kb = args
                    eng.dma_start(
                        w_tiles[db][:, ka * P:kb * P],
                        w_d[db * P:(db + 1) * P, ka * P:kb * P],
                    )
                else:
                    eng.dma_start(bt[:], bias_d[:])

            # Warm the PE's p-state ramp while the first tiles land.
            warm = ppool.tile([P, P], dt.bfloat16, name="warm", tag="warm")
            nc.vector.memset(warm[:], 0.0)
            wps = pspool.tile([P, P], dt.float32, name="wps", tag="ps")
            for _ in range(16):
                nc.tensor.matmul(wps[:], warm[:], warm[:], start=True, stop=True)

            psums = {}

            def alloc_db(db):
                psums[db] = [
                    pspool.tile([P, cw], dt.float32, name=f"ps{db}_{j}", tag="ps")
                    for j, (off, cw) in enumerate(chunks)
                ]

            def emit_mms(db, ki):
                lhsT = w_tiles[db][:, ki * P:(ki + 1) * P]
                for j, (off, cw) in enumerate(chunks):
                    nc.tensor.matmul(
                        psums[db][j][:, :cw],
                        lhsT,
                        x_tiles[ki][:, off:off + cw],
                        start=(ki == 0),
                        stop=(ki == KT - 1),
                    )

            ep = 0

            def evac_db(db, last=False):
                nonlocal ep
                ysb = ypool.tile([P, C2], dt.bfloat16, name="ysb", tag="ysb")
                bias_col = bt[:, db:db + 1]
                for j, (off, cw) in enumerate(chunks):
                    if ep % 2 == 0:
                        nc.scalar.add(
                            ysb[:, off:off + cw], psums[db][j][:, :cw], bias_col
                        )
                    else:
                        nc.vector.tensor_scalar_add(
                            ysb[:, off:off + cw], psums[db][j][:, :cw], bias_col
                        )
                    ep += 1
                eng = nc.sync if db % 2 == 0 else nc.scalar
                eng.dma_start(y_d[:, db, 0:C2], ysb[:])

            # wave0: first WAVE0 blocks interleaved per k-step
            for db in range(WAVE0):
                alloc_db(db)
            for ki in range(KT):
                for db in range(WAVE0):
                    emit_mms(db, ki)
            for db in range(WAVE0):
                evac_db(db)
            # remaining blocks one at a time
            for db in range(WAVE0, DB):
                alloc_db(db)
                for ki in range(KT):
                    emit_mms(db, ki)
                evac_db(db)

            # leftover phase: columns [C2, C) for all blocks, 8 at a time
            if CL:
                for g in range(0, DB, 8):
                    nds = list(range(g, min(g + 8, DB)))
                    lps = {
                        db: pspool.tile(
                            [P, CL], dt.float32, name=f"lp{db}", tag="ps"
                        )
                        for db in nds
                    }
                    for ki in range(KT):
                        lhs_x = x_tiles[ki][:, C2:C]
                        for db in nds:
                            nc.tensor.matmul(
                                lps[db][:],
                                w_tiles[db][:, ki * P:(ki + 1) * P],
                                lhs_x,
                                start=(ki == 0),
                                stop=(ki == KT - 1),
                            )
                    strip = ypool.tile(
                        [P, len(nds), CL], dt.bfloat16, name=f"strip{g}", tag="strip"
                    )
                    for i, db in enumerate(nds):
                        if ep % 2 == 0:
                            nc.scalar.add(
                                strip[:, i, :], lps[db][:], bt[:, db:db + 1]
                            )
                        else:
                            nc.vector.tensor_scalar_add(
                                strip[:, i, :], lps[db][:], bt[:, db:db + 1]
                            )
                        ep += 1
                    eng = nc.sync if (g // 8) % 2 == 0 else nc.scalar
                    eng.dma_start(y_d[:, g:g + len(nds), C2:C], strip[:])

    nc.compile()
    return nc


def _run_spmd(in_maps, C, trace=False, trace_cores=None):
    from concourse.bass_utils import run_bass_kernel_spmd

    nc = _nc_cache.get(C)
    if nc is None:
        nc = _build_bass(C)
        _nc_cache[C] = nc
    return run_bass_kernel_spmd(
        nc,
        in_maps,
        core_ids=list(range(N_CORES)),
        trace=trace,
        trace_cores=trace_cores,
    )


def _prepare(x, weight, bias, ids_sorted):
    """Host-side routing: returns (in_maps, C, counts, starts)."""
    x = np.asarray(x)
    weight = np.asarray(weight)
    bias = np.asarray(bias)
    ids = np.asarray(ids_sorted)

    counts = np.bincount(ids, minlength=N_EXP).astype(np.int64)
    starts = np.zeros(N_EXP, dtype=np.int64)
    starts[1:] = np.cumsum(counts)[:-1]
    C = max(int(counts.max()), 2)
    C += C % 2

    xb = x.astype(BF16)
    in_maps = []
    for e in range(N_EXP):
        n_e = int(counts[e])
        xeT = np.zeros((D_IN, C), dtype=BF16)
        if n_e:
            xeT[:, :n_e] = xb[starts[e]:starts[e] + n_e].T
        # db-major weight: row (db*128+p) = [w.T[kt*128+p, db*128+d] for kt, d]
        weT = weight[e].T.astype(BF16)  # [d_in, d_out]
        wdb = np.ascontiguousarray(
            weT.reshape(D_IN // P, P, D_OUT // P, P).transpose(2, 1, 0, 3)
        ).reshape(D_OUT, D_IN)
        bp = np.ascontiguousarray(
            bias[e].astype(BF16).astype(np.float32).reshape(D_OUT // P, P).T
        )
        in_maps.append({"xT": xeT, "wdb": wdb, "biasp": bp})
    return in_maps, C, counts, starts


def _assemble(results, counts, starts):
    out = np.empty((N_TOK, D_OUT), dtype=BF16)
    for e in range(N_EXP):
        n_e = int(counts[e])
        if n_e:
            # yT is [p, db, c] -> [d_out, c]
            yT = results[e]["yT"].transpose(1, 0, 2).reshape(D_OUT, -1)
            out[starts[e]:starts[e] + n_e] = yT[:, :n_e].T
    return out


def kernel(x, weight, bias, ids_sorted):
    in_maps, C, counts, starts = _prepare(x, weight, bias, ids_sorted)
    res = _run_spmd(in_maps, C)
    return _assemble(res.results, counts, starts)


# revision 3
# speedup vs baseline: 1.0166x; 1.0166x over previous
"""MoE grouped linear (DMoELinear) on 8 Trainium2 NeuronCores.

Expert-parallel sharding: tokens are sorted by expert id, so expert e's
tokens form one contiguous slice. Core e receives expert e's tokens
(padded to a uniform capacity C = max group size, so all cores run one
SPMD NEFF), expert e's weight and bias, and computes
    yT_e = (x_e @ W_e.T).T.bf16 + b_e.bf16
with the weight block as the stationary matmul operand and tokens as
the moving free dim. Per-core structure:

  - Output blocks db0/db1 interleave per k-step during the x stream-in
    (the PE consumes each freshly-landed x k-tile twice, ~1.3us, while
    the next tile arrives in ~0.9us across both rings - no starvation);
    blocks 2..15 then run one at a time, three PSUM chunks each.
  - Input DMAs are hand-ordered by PE need time: tiny k0 slices of
    w0/w1 lead each ring, whole x k-tiles alternate between rings, the
    rest of w0/w1 in two pieces between them, bulk w blocks after.
    Fewer, larger DMAs matter: each submit costs ~0.65us of engine time
    and semaphore reuse throttles deep queues.
  - A bridged PE warmup (dummy matmuls on a zeroed tile) keeps the
    tensor clock ramping from body start until the first real matmul;
    any idle gap resets the p-state timer.
  - The last block runs chunk-major so its two big output chunks are
    evacuated and in flight before the final 58-column chunk finishes;
    the kernel ends on a 15KB DMA instead of a 280KB one.

The bias add fuses into the PSUM-evacuation op as a per-partition
scalar. The host does all routing/gather in numpy.
"""

import numpy as np
import ml_dtypes

N_TOK, D_IN, D_OUT, N_EXP = 8192, 1024, 2048, 8
N_CORES = 8
P = 128
NFREE = 512  # max matmul moving free dim (one PSUM bank of f32)

BF16 = ml_dtypes.bfloat16

_nc_cache: dict[int, object] = {}

KT = D_IN // P   # 8 contraction tiles
DB = D_OUT // P  # 16 output-row blocks
N_WARM = 24


def _chunks(C):
    out = []
    off = 0
    while off < C:
        cw = min(NFREE, C - off)
        out.append((off, cw))
        off += cw
    return out


def _build_bass(C: int):
    """Emit the per-core Bass/Tile kernel for token capacity C."""
    import concourse.bass as bass  # noqa: F401  (registers engines)
    import concourse.mybir as mybir
    import concourse.tile as tile
    from concourse import bacc

    dt = mybir.dt
    chunks = _chunks(C)

    nc = bacc.Bacc("TRN2", target_bir_lowering=False)

    xT_d = nc.dram_tensor("xT", [D_IN, C], dt.bfloat16, kind="ExternalInput")
    # w: db-major, partition-contiguous: row (db*128+p) holds all KT
    # 128-wide k-slices for output block db, SBUF partition p.
    w_d = nc.dram_tensor("wdb", [DB * P, KT * P], dt.bfloat16, kind="ExternalInput")
    bias_d = nc.dram_tensor("biasp", [P, DB], dt.float32, kind="ExternalInput")
    y_d = nc.dram_tensor("yT", [D_OUT, C], dt.bfloat16, kind="ExternalOutput")

    with tile.TileContext(nc) as tc:
        with (
            tc.tile_pool(name="persist", bufs=1) as ppool,
            tc.tile_pool(name="yout", bufs=3) as ypool,
            tc.tile_pool(name="psum", bufs=8, space="PSUM") as pspool,
        ):
            x_tiles = [
                ppool.tile([P, C], dt.bfloat16, name=f"x{ki}", tag=f"x{ki}")
                for ki in range(KT)
            ]
            w_tiles = [
                ppool.tile([P, KT * P], dt.bfloat16, name=f"w{db}", tag=f"w{db}")
                for db in range(DB)
            ]
            bt = ppool.tile([P, DB], dt.float32, name="bias", tag="bias")

            def xdma(eng, ki):
                eng.dma_start(x_tiles[ki][:], xT_d[ki * P:(ki + 1) * P, :])

            def wdma(eng, db, ka=0, kb=KT):
                eng.dma_start(
                    w_tiles[db][:, ka * P:kb * P],
                    w_d[db * P:(db + 1) * P, ka * P:kb * P],
                )

            # sync ring: w0 pieces + even x tiles + w2 + bias + odd w blocks
            # scalar ring: w1 pieces + odd x tiles + w3/w4 + even w blocks
            wdma(nc.sync, 0, 0, 1)
            wdma(nc.scalar, 1, 0, 1)
            xdma(nc.sync, 0)
            xdma(nc.scalar, 1)
            wdma(nc.sync, 0, 1, 4)
            wdma(nc.scalar, 1, 1, 4)
            xdma(nc.sync, 2)
            xdma(nc.scalar, 3)
            wdma(nc.sync, 0, 4, KT)
            wdma(nc.scalar, 1, 4, KT)
            xdma(nc.sync, 4)
            xdma(nc.scalar, 5)
            wdma(nc.sync, 2)
            wdma(nc.scalar, 3)
            xdma(nc.sync, 6)
            xdma(nc.scalar, 7)
            nc.sync.dma_start(bt[:], bias_d[:])
            wdma(nc.scalar, 4)
            for db in range(5, DB):
                wdma(nc.sync if db % 2 == 1 else nc.scalar, db)

            # Warm the PE p-state ramp; must bridge to the first real
            # matmul without an idle gap (idle resets the ramp timer).
            warm = ppool.tile([P, P], dt.bfloat16, name="warm", tag="warm")
            nc.vector.memset(warm[:], 0.0)
            wps = pspool.tile([P, P], dt.float32, name="wps", tag="ps")
            for _ in range(N_WARM):
                nc.tensor.matmul(wps[:], warm[:], warm[:], start=True, stop=True)

            psums = {}

            def alloc_db(db):
                psums[db] = [
                    pspool.tile([P, cw], dt.float32, name=f"ps{db}_{j}", tag="ps")
                    for j, (off, cw) in enumerate(chunks)
                ]

            def emit_mms(db, ki):
                lhsT = w_tiles[db][:, ki * P:(ki + 1) * P]
                for j, (off, cw) in enumerate(chunks):
                    nc.tensor.matmul(
                        psums[db][j][:, :cw],
                        lhsT,
                        x_tiles[ki][:, off:off + cw],
                        start=(ki == 0),
                        stop=(ki == KT - 1),
                    )

            ep = 0

            def evac_chunk(db, j, ysb):
                nonlocal ep
                off, cw = chunks[j]
                bias_col = bt[:, db:db + 1]
                if ep % 2 == 0:
                    nc.scalar.add(ysb[:, off:off + cw], psums[db][j][:, :cw], bias_col)
                else:
                    nc.vector.tensor_scalar_add(
                        ysb[:, off:off + cw], psums[db][j][:, :cw], bias_col
                    )
                ep += 1

            def evac_db(db, split_dma):
                ysb = ypool.tile([P, C], dt.bfloat16, name="ysb", tag="ysb")
                for j in range(len(chunks)):
                    evac_chunk(db, j, ysb)
                if split_dma:
                    for j, (off, cw) in enumerate(chunks):
                        eng = nc.sync if (db + j) % 2 == 0 else nc.scalar
                        eng.dma_start(
                            y_d[db * P:(db + 1) * P, off:off + cw],
                            ysb[:, off:off + cw],
                        )
                else:
                    eng = nc.sync if db % 2 == 0 else nc.scalar
                    eng.dma_start(y_d[db * P:(db + 1) * P, :], ysb[:])

            # wave0: db0/db1 interleaved per k-step during the x trickle
            alloc_db(0)
            alloc_db(1)
            for ki in range(KT):
                emit_mms(0, ki)
                emit_mms(1, ki)
            evac_db(0, split_dma=False)
            evac_db(1, split_dma=False)

            for db in range(2, DB - 1):
                alloc_db(db)
                for ki in range(KT):
                    emit_mms(db, ki)
                evac_db(db, split_dma=(db >= DB - 2))

            # last block chunk-major: big chunks finish, evacuate, and
            # start their DMAs while the PE runs the final small chunk.
            db = DB - 1
            alloc_db(db)
            ysb = ypool.tile([P, C], dt.bfloat16, name="ysb_last", tag="ysb")
            for j, (off, cw) in enumerate(chunks):
                for ki in range(KT):
                    nc.tensor.matmul(
                        psums[db][j][:, :cw],
                        w_tiles[db][:, ki * P:(ki + 1) * P],
                        x_tiles[ki][:, off:off + cw],
                        start=(ki == 0),
                        stop=(ki == KT - 1),
                    )
                evac_chunk(db, j, ysb)
                eng = nc.sync if j % 2 == 0 else nc.scalar
                eng.dma_start(
                    y_d[db * P:(db + 1) * P, off:off + cw], ysb[:, off:off + cw]
                )

    nc.compile()
    return nc


def _run_spmd(in_maps, C, trace=False, trace_cores=None):
    from concourse.bass_utils import run_bass_kernel_spmd

    nc = _nc_cache.get(C)
    if nc is None:
        nc = _build_bass(C)
        _nc_cache[C] = nc
    return run_bass_kernel_spmd(
        nc,
        in_maps,
        core_ids=list(range(N_CORES)),
        trace=trace,
        trace_cores=trace_cores,
    )


def _prepare(x, weight, bias, ids_sorted):
    """Host-side routing: returns (in_maps, C, counts, starts)."""
    x = np.asarray(x)
    weight = np.asarray(weight)
    bias = np.asarray(bias)
    ids = np.asarray(ids_sorted)

    counts = np.bincount(ids, minlength=N_EXP).astype(np.int64)
    starts = np.zeros(N_EXP, dtype=np.int64)
    starts[1:] = np.cumsum(counts)[:-1]
    C = max(int(counts.max()), 2)
    C += C % 2

    xb = x.astype(BF16)
    in_maps = []
    for e in range(N_EXP):
        n_e = int(counts[e])
        xeT = np.zeros((D_IN, C), dtype=BF16)
        if n_e:
            xeT[:, :n_e] = xb[starts[e]:starts[e] + n_e].T
        # db-major weight: row (db*128+p) = [w.T[kt*128+p, db*128+d] for kt, d]
        weT = weight[e].T.astype(BF16)  # [d_in, d_out]
        wdb = np.ascontiguousarray(
            weT.reshape(D_IN // P, P, D_OUT // P, P).transpose(2, 1, 0, 3)
        ).reshape(D_OUT, D_IN)
        bp = np.ascontiguousarray(
            bias[e].astype(BF16).astype(np.float32).reshape(D_OUT // P, P).T
        )
        in_maps.append({"xT": xeT, "wdb": wdb, "biasp": bp})
    return in_maps, C, counts, starts


def _assemble(results, counts, starts):
    out = np.empty((N_TOK, D_OUT), dtype=BF16)
    for e in range(N_EXP):
        n_e = int(counts[e])
        if n_e:
            out[starts[e]:starts[e] + n_e] = results[e]["yT"][:, :n_e].T
    return out


def kernel(x, weight, bias, ids_sorted):
    in_maps, C, counts, starts = _prepare(x, weight, bias, ids_sorted)
    res = _run_spmd(in_maps, C)
    return _assemble(res.results, counts, starts)


# revision 4
# speedup vs baseline: 1.0591x; 1.0418x over previous
"""MoE grouped linear (DMoELinear) on 8 Trainium2 NeuronCores.

Expert-parallel sharding: tokens are sorted by expert id, so expert e's
tokens form one contiguous slice. Core e receives expert e's tokens
(padded to a uniform capacity C = max group size, so all cores run one
SPMD NEFF), expert e's weight and bias, and computes
    yT_e = (x_e @ W_e.T).T.bf16 + b_e.bf16
with the weight block as the stationary matmul operand and tokens as
the moving free dim. Per-core structure:

  - wave0: output blocks 0-3 interleave per k-step over columns
    [0, 1024) as two 512-col PSUM banks each (4 blocks x 2 = all 8
    banks; the PE warmup matmuls retarget block 0's first bank, which
    the first real start=True matmul resets). The PE consumes each
    freshly-landed x k-tile four times (~1.7us) while the next tile
    arrives (~1.3us effective) - no starvation during the x stream-in,
    which is latency-bound at ~90-110 GB/s/ring for the first tiles.
  - blocks 0-3's leftover columns [1024, C) run as a small 4-bank
    strip right after wave0 (x is fully resident by then), written out
    mid-stream where the DMA latency is hidden.
  - blocks 4..14 run one at a time with three PSUM chunks
    [512, 512, C-1024]; the last block runs chunk-major so its two big
    output chunks are evacuated and in flight before the final
    58-column chunk finishes - the kernel ends on a 15KB DMA.
  - Input DMAs are emitted in PE need-time order via a greedy two-ring
    assignment (fine k-slices of wave0's weights + whole x tiles lead,
    bulk weight blocks follow). Fewer, larger DMAs matter: each submit
    costs ~0.65us of engine time and ~128 descriptors of ~20ns.
  - A bridged PE warmup keeps the tensor-clock p-state ramping from
    body start until the first real matmul (an idle PE resets it).

The bias add fuses into the PSUM-evacuation op as a per-partition
scalar. The host does all routing/gather in numpy.
"""

import numpy as np
import ml_dtypes

N_TOK, D_IN, D_OUT, N_EXP = 8192, 1024, 2048, 8
N_CORES = 8
P = 128
NFREE = 512  # max matmul moving free dim (one PSUM bank of f32)

BF16 = ml_dtypes.bfloat16

_nc_cache: dict[int, object] = {}

KT = D_IN // P   # 8 contraction tiles
DB = D_OUT // P  # 16 output-row blocks
WAVE0 = 4        # blocks interleaved during the x stream-in
N_WARM = 24


def _plan_dmas(C):
    """Greedy two-ring schedule of input pieces by PE need-time (us)."""
    bw = 110e3   # bytes per us per ring (early, latency-bound)
    sub = 0.55   # per-submit engine/serialization cost
    kper = 0.43 * WAVE0
    items = []
    items.append((0.0, P * C * 2, "x", (0,)))
    for ki in range(1, KT):
        items.append((kper * ki, P * C * 2, "x", (ki,)))
    for db in range(WAVE0):
        for (ka, kb) in ((0, 1), (1, 4), (4, KT)):
            items.append(
                (kper * ka + 0.43 * db, P * (kb - ka) * P * 2, "w", (db, ka, kb))
            )
    t_wave0 = kper * KT
    for db in range(WAVE0, DB):
        items.append(
            (t_wave0 + 2.0 + (db - WAVE0) * 0.43 * 3, P * D_IN * 2, "w", (db, 0, KT))
        )
    items.append((t_wave0, P * DB * 4, "bias", ()))

    items.sort(key=lambda it: it[0])
    ring_t = [0.0, 0.0]
    order = []
    for need, nbytes, kind, args in items:
        r = 0 if ring_t[0] <= ring_t[1] else 1
        ring_t[r] += sub + nbytes / bw
        order.append((r, kind, args))
    return order


def _build_bass(C: int):
    """Emit the per-core Bass/Tile kernel for token capacity C."""
    import concourse.bass as bass  # noqa: F401  (registers engines)
    import concourse.mybir as mybir
    import concourse.tile as tile
    from concourse import bacc

    dt = mybir.dt
    C2 = min(C, 2 * NFREE)     # wave0 columns (2 psum banks per block)
    CL = C - C2                # leftover columns
    w0chunks = [(0, min(NFREE, C2))]
    if C2 > NFREE:
        w0chunks.append((NFREE, C2 - NFREE))
    chunks = list(w0chunks)
    if CL:
        chunks.append((C2, CL))

    nc = bacc.Bacc("TRN2", target_bir_lowering=False)

    xT_d = nc.dram_tensor("xT", [D_IN, C], dt.bfloat16, kind="ExternalInput")
    # w: db-major, partition-contiguous: row (db*128+p) holds all KT
    # 128-wide k-slices for output block db, SBUF partition p.
    w_d = nc.dram_tensor("wdb", [DB * P, KT * P], dt.bfloat16, kind="ExternalInput")
    bias_d = nc.dram_tensor("biasp", [P, DB], dt.float32, kind="ExternalInput")
    y_d = nc.dram_tensor("yT", [D_OUT, C], dt.bfloat16, kind="ExternalOutput")

    with tile.TileContext(nc) as tc:
        with (
            tc.tile_pool(name="persist", bufs=1) as ppool,
            tc.tile_pool(name="yout", bufs=3) as ypool,
            tc.tile_pool(name="psum", bufs=8, space="PSUM") as pspool,
        ):
            x_tiles = [
                ppool.tile([P, C], dt.bfloat16, name=f"x{ki}", tag=f"x{ki}")
                for ki in range(KT)
            ]
            w_tiles = [
                ppool.tile([P, KT * P], dt.bfloat16, name=f"w{db}", tag=f"w{db}")
                for db in range(DB)
            ]
            bt = ppool.tile([P, DB], dt.float32, name="bias", tag="bias")

            for r, kind, args in _plan_dmas(C):
                eng = nc.sync if r == 0 else nc.scalar
                if kind == "x":
                    (ki,) = args
                    eng.dma_start(x_tiles[ki][:], xT_d[ki * P:(ki + 1) * P, :])
                elif kind == "w":
                    db, ka, kb = args
                    eng.dma_start(
                        w_tiles[db][:, ka * P:kb * P],
                        w_d[db * P:(db + 1) * P, ka * P:kb * P],
                    )
                else:
                    eng.dma_start(bt[:], bias_d[:])

            psums = {}

            def alloc_db(db, chs):
                psums[db] = [
                    pspool.tile([P, cw], dt.float32, name=f"ps{db}_{j}", tag="ps")
                    for j, (off, cw) in enumerate(chs)
                ]

            # wave0 psums first so the warmup can target a real bank
            for db in range(WAVE0):
                alloc_db(db, w0chunks)

            # Warmup matmuls write block 0's first bank; the first real
            # start=True matmul resets it. Must bridge to the first real
            # matmul without an idle gap (idle resets the ramp timer).
            warm = ppool.tile([P, P], dt.bfloat16, name="warm", tag="warm")
            nc.vector.memset(warm[:], 0.0)
            for _ in range(N_WARM):
                nc.tensor.matmul(
                    psums[0][0][:, :P], warm[:], warm[:], start=True, stop=True
                )

            def emit_mms(db, ki, chs):
                lhsT = w_tiles[db][:, ki * P:(ki + 1) * P]
                for j, (off, cw) in enumerate(chs):
                    nc.tensor.matmul(
                        psums[db][j][:, :cw],
                        lhsT,
                        x_tiles[ki][:, off:off + cw],
                        start=(ki == 0),
                        stop=(ki == KT - 1),
                    )

            ep = 0

            def evac_chunk(db, j, off, cw, ysb, yoff):
                nonlocal ep
                bias_col = bt[:, db:db + 1]
                if ep % 2 == 0:
                    nc.scalar.add(ysb[:, yoff:yoff + cw], psums[db][j][:, :cw], bias_col)
                else:
                    nc.vector.tensor_scalar_add(
                        ysb[:, yoff:yoff + cw], psums[db][j][:, :cw], bias_col
                    )
                ep += 1

            # wave0: blocks 0..3 interleaved per k-step over [0, C2)
            for ki in range(KT):
                for db in range(WAVE0):
                    emit_mms(db, ki, w0chunks)
            for db in range(WAVE0):
                ysb = ypool.tile([P, C2], dt.bfloat16, name="ysb", tag="ysb")
                for j, (off, cw) in enumerate(w0chunks):
                    evac_chunk(db, j, off, cw, ysb, off)
                eng = nc.sync if db % 2 == 0 else nc.scalar
                eng.dma_start(y_d[db * P:(db + 1) * P, 0:C2], ysb[:])

            # leftover strip for wave0 blocks: columns [C2, C)
            if CL:
                lps = {
                    db: pspool.tile([P, CL], dt.float32, name=f"lp{db}", tag="ps")
                    for db in range(WAVE0)
                }
                for ki in range(KT):
                    xl = x_tiles[ki][:, C2:C]
                    for db in range(WAVE0):
                        nc.tensor.matmul(
                            lps[db][:],
                            w_tiles[db][:, ki * P:(ki + 1) * P],
                            xl,
                            start=(ki == 0),
                            stop=(ki == KT - 1),
                        )
                for db in range(WAVE0):
                    lsb = ypool.tile([P, CL], dt.bfloat16, name=f"lsb{db}", tag="lsb")
                    if ep % 2 == 0:
                        nc.scalar.add(lsb[:], lps[db][:], bt[:, db:db + 1])
                    else:
                        nc.vector.tensor_scalar_add(lsb[:], lps[db][:], bt[:, db:db + 1])
                    ep += 1
                    eng = nc.sync if db % 2 == 0 else nc.scalar
                    eng.dma_start(y_d[db * P:(db + 1) * P, C2:C], lsb[:])

            # blocks WAVE0..14: one at a time, all chunks
            for db in range(WAVE0, DB - 1):
                alloc_db(db, chunks)
                for ki in range(KT):
                    emit_mms(db, ki, chunks)
                ysb = ypool.tile([P, C], dt.bfloat16, name="ysb", tag="ysb")
                for j, (off, cw) in enumerate(chunks):
                    evac_chunk(db, j, off, cw, ysb, off)
                eng = nc.sync if db % 2 == 0 else nc.scalar
                eng.dma_start(y_d[db * P:(db + 1) * P, :], ysb[:])

            # last block chunk-major: big chunks finish, evacuate, and
            # start their DMAs while the PE runs the final small chunk.
            db = DB - 1
            alloc_db(db, chunks)
            ysb = ypool.tile([P, C], dt.bfloat16, name="ysb_last", tag="ysb")
            for j, (off, cw) in enumerate(chunks):
                for ki in range(KT):
                    nc.tensor.matmul(
                        psums[db][j][:, :cw],
                        w_tiles[db][:, ki * P:(ki + 1) * P],
                        x_tiles[ki][:, off:off + cw],
                        start=(ki == 0),
                        stop=(ki == KT - 1),
                    )
                evac_chunk(db, j, off, cw, ysb, off)
                eng = nc.sync if j % 2 == 0 else nc.scalar
                eng.dma_start(
                    y_d[db * P:(db + 1) * P, off:off + cw], ysb[:, off:off + cw]
                )

    nc.compile()
    return nc


def _run_spmd(in_maps, C, trace=False, trace_cores=None):
    from concourse.bass_utils import run_bass_kernel_spmd

    nc = _nc_cache.get(C)
    if nc is None:
        nc = _build_bass(C)
        _nc_cache[C] = nc
    return run_bass_kernel_spmd(
        nc,
        in_maps,
        core_ids=list(range(N_CORES)),
        trace=trace,
        trace_cores=trace_cores,
    )


def _prepare(x, weight, bias, ids_sorted):
    """Host-side routing: returns (in_maps, C, counts, starts)."""
    x = np.asarray(x)
    weight = np.asarray(weight)
    bias = np.asarray(bias)
    ids = np.asarray(ids_sorted)

    counts = np.bincount(ids, minlength=N_EXP).astype(np.int64)
    starts = np.zeros(N_EXP, dtype=np.int64)
    starts[1:] = np.cumsum(counts)[:-1]
    C = max(int(counts.max()), 2)
    C += C % 2

    xb = x.astype(BF16)
    in_maps = []
    for e in range(N_EXP):
        n_e = int(counts[e])
        xeT = np.zeros((D_IN, C), dtype=BF16)
        if n_e:
            xeT[:, :n_e] = xb[starts[e]:starts[e] + n_e].T
        # db-major weight: row (db*128+p) = [w.T[kt*128+p, db*128+d] for kt, d]
        weT = weight[e].T.astype(BF16)  # [d_in, d_out]
        wdb = np.ascontiguousarray(
            weT.reshape(D_IN // P, P, D_OUT // P, P).transpose(2, 1, 0, 3)
        ).reshape(D_OUT, D_IN)
        bp = np.ascontiguousarray(
            bias[e].astype(BF16).astype(np.float32).reshape(D_OUT // P, P).T
        )
        in_maps.append({"xT": xeT, "wdb": wdb, "biasp": bp})
    return in_maps, C, counts, starts


def _assemble(results, counts, starts):
    out = np.empty((N_TOK, D_OUT), dtype=BF16)
    for e in range(N_EXP):
        n_e = int(counts[e])
        if n_e:
            out[starts[e]:starts[e] + n_e] = results[e]["yT"][:, :n_e].T
    return out


def kernel(x, weight, bias, ids_sorted):
    in_maps, C, counts, starts = _prepare(x, weight, bias, ids_sorted)
    res = _run_spmd(in_maps, C)
    return _assemble(res.results, counts, starts)
